# revision 21
# baseline (speedup 1.0000x reference)
"""Trainium2 Bass kernel for nn_Atom_57732950393048 (Nucleus MLP + RoPE).

Math (per batch b, all features f, tokens n):
    y = x @ W^T + phase                      # [N, 512], W = perm_freqs
    s = sin(y)            in [-1, 1]
    u = sigmoid(s)        in [0.2689, 0.7311]
    val = sum_k w_k relu(u - k/15) + bias,   w = softplus(spline_heights)
    out = rope(val)

Because u is confined to [sigmoid(-1), sigmoid(1)], spline bins 0..4 are
always active (linear) and bins 11..15 always inactive.  With the tanh form
u = 0.5 + 0.5 t, t = tanh(s/2):
    val = sum_{k=5..10} wb_k * max(t, g'_k) + ab * t + bb
with wb_k = w_k/2, g'_k = 2k/15 - 1, ab = sum_{k<=4} w_k / 2, and all the
constants folded into bb.  max(t, g'_k) == max(t, -1) == t for the linear
term, so the linear term is just one more diagonal matmul.

Device layout (one batch per core, 8 cores, data parallel):
  - orientation "B": features on partitions (4 blocks of 128), tokens on the
    free dim.  Main matmul: out[f, m] += WT_chunk.T @ XT_chunk, f32r.
  - spline accumulation: PE diagonal matmuls  val[f, m] += diag(wb_k) @ t_k
    accumulated in PSUM (t_k = max(t, g'_k) computed by DVE tensor_scalar).
  - RoPE applied in orientation B on feature-PERMUTED channels (evens then
    odds, so rope partners are partition blocks (0,2) and (1,3)); the
    per-feature constant bb is folded into the rope multiplies via
    scalar_tensor_tensor: (val + bb) * table.
  - PE transposes (f32r) flip [f, m] -> [m, f] and ACT copies interleave
    even/odd channels into the final [m, 512] layout, then DMA to DRAM.
"""

import numpy as np


def ml_dtypes_bfloat16():
    import ml_dtypes

    return ml_dtypes.bfloat16


NUM_BINS = 16
DAY_LENGTH = 64
B, N, IN_DIM, DIM = 8, 2048, 768, 512
NCORES = 8

_CACHE = {}
TRACE = False
MAIN_BF16 = False
ACT_BINS = (0, 1)      # bins on ScalarE (relu form)
GPSIMD_BINS = ()       # bins on GpSimd (measured: disastrous)
GPSIMD_ROPE_TAIL = False


def _build():
    import concourse.bacc as bacc
    import concourse.tile as tile
    from concourse import mybir

    # Pin all our activation funcs to the single table set that holds every
    # one of them (silu_and_others: sin + tanh + relu + copy + identity) by
    # hiding those funcs from every other set in the selection table.  Set
    # ids are positional, so membership may be edited but never reordered.
    import concourse.hw_specs as hw_specs

    _orig_tables = hw_specs.get_activation_tables

    def _pinned_tables(arch):
        t = _orig_tables(arch)
        A = mybir.ActivationFunctionType
        shared = {A.Sin, A.Tanh, A.Copy, A.Identity, A.Relu}
        if "silu_and_others" in t and shared <= t["silu_and_others"]:
            for name in t:
                if name != "silu_and_others":
                    t[name] = t[name] - shared
        return t

    hw_specs.get_activation_tables = _pinned_tables
    bacc.get_activation_tables = _pinned_tables

    F32 = mybir.dt.float32
    F32R = mybir.dt.float32r
    Alu = mybir.AluOpType
    Act = mybir.ActivationFunctionType

    nc = bacc.Bacc(trn_type="TRN2")

    BF16 = mybir.dt.bfloat16
    MDT = BF16 if MAIN_BF16 else F32R
    xt = nc.dram_tensor("xt", [128, 6, N], MDT, kind="ExternalInput")
    wt = nc.dram_tensor("wt", [128, 6, DIM], MDT, kind="ExternalInput")
    scal = nc.dram_tensor("scal", [128, 64], F32, kind="ExternalInput")
    ident = nc.dram_tensor("ident", [128, 128], F32R, kind="ExternalInput")
    identb = nc.dram_tensor("identb", [128, 128], BF16, kind="ExternalInput")
    cosT = nc.dram_tensor("cosT", [256, N], F32, kind="ExternalInput")
    sinT = nc.dram_tensor("sinT", [256, N], F32, kind="ExternalInput")
    out = nc.dram_tensor("out", [N, DIM], F32, kind="ExternalOutput")

    # bin breakpoints in t-space; the 7th entry (-1.0) implements the linear
    # term: max(t, -1) == t.
    GPRIME = [2.0 * k / 15.0 - 1.0 for k in range(5, 11)] + [-1.0]

    MBLK = N // 512  # 4 m-blocks of 512 tokens

    def bass_ap_chunks(ap, nk, width):
        # [nk*128, width] DRAM view -> [128 p, nk, width] (partition-major)
        return ap.rearrange("(k p) w -> p k w", p=128)

    with tile.TileContext(nc) as tc:
        from contextlib import ExitStack

        with ExitStack() as ctx:
            res = ctx.enter_context(tc.tile_pool(name="res", bufs=1))
            xtp = ctx.enter_context(tc.tile_pool(name="xtp", bufs=2))
            sbw = ctx.enter_context(tc.tile_pool(name="sbw", bufs=4))
            tkp = ctx.enter_context(tc.tile_pool(name="tkp", bufs=3))
            rop = ctx.enter_context(tc.tile_pool(name="rop", bufs=3))
            outp = ctx.enter_context(tc.tile_pool(name="outp", bufs=2))
            ps_y = ctx.enter_context(tc.tile_pool(name="ps_y", bufs=3, space="PSUM"))
            ps_v = ctx.enter_context(tc.tile_pool(name="ps_v", bufs=3, space="PSUM"))
            ps_o = ctx.enter_context(tc.tile_pool(name="ps_o", bufs=2, space="PSUM"))

            # --- resident tensors.  DMA order is startup-latency-critical:
            # the first k-chunks of x and W go first (the first matmuls gate
            # on them), then the merged scalar tensor, identity, tables. ---
            wt_s = res.tile([128, 6, DIM], MDT, tag="wt")
            scal_s = res.tile([128, 64], F32, tag="scal")
            wb_sc = scal_s[:, 0:28]
            nb_sc = scal_s[:, 28:56]
            ph_s = scal_s[:, 56:60]
            bb_s = scal_s[:, 60:64]
            ident_s = res.tile([128, 128], F32R, tag="ident")
            identb_s = res.tile([128, 128], BF16, tag="identb")
            cos_s = res.tile([128, 2, N], F32, tag="cos")
            sin_s = res.tile([128, 2, N], F32, tag="sin")

            # one-iteration software pipeline: the transpose/interleave/DMA
            # tail of iteration i is emitted after iteration i+1's first main
            # matmul group so PE never drains while DVE finishes the rope.
            pending = []

            def flush_tail():
                for fn in pending:
                    fn()
                pending.clear()

            mb_out_ts = {}
            for it in range(2 * MBLK):
                mb, pb = divmod(it, 2)
                fba, fbb = (0, 2) if pb == 0 else (1, 3)

                if pb == 0:
                    xt_t = xtp.tile([128, 6, 512], MDT, tag="xt")
                    if mb == 0:
                        for k in range(6):
                            nc.sync.dma_start(out=xt_t[:, k, :], in_=xt[:, k, 0:512])
                            nc.sync.dma_start(out=wt_s[:, k, :], in_=wt[:, k, :])
                            if k == 0:
                                nc.sync.dma_start(out=scal_s, in_=scal[:])
                            elif k == 1:
                                nc.sync.dma_start(out=ident_s, in_=ident[:])
                            elif k == 2:
                                nc.sync.dma_start(out=identb_s, in_=identb[:])
                        nc.sync.dma_start(out=cos_s[:, 0, :], in_=cosT[0:128, :])
                        nc.sync.dma_start(out=sin_s[:, 0, :], in_=sinT[0:128, :])
                    else:
                        nc.sync.dma_start(
                            out=xt_t, in_=xt[:, :, mb * 512:(mb + 1) * 512]
                        )
                    mb_out_ts[mb] = []
                    for mc in range(4):
                        o_t = outp.tile([128, DIM], F32, tag=f"out{mc}")
                        mb_out_ts[mb].append(o_t)
                    if mb:
                        mb_out_ts.pop(mb - 1)
                    xt_cur = xt_t
                else:
                    xt_t = xt_cur
                    if mb == 0:
                        # pair-1 tables load during pair-0 compute
                        nc.sync.dma_start(out=cos_s[:, 1, :], in_=cosT[128:256, :])
                        nc.sync.dma_start(out=sin_s[:, 1, :], in_=sinT[128:256, :])
                out_ts = mb_out_ts[mb]

                vals = []
                for fi, fb in enumerate((fba, fbb)):
                    y = ps_y.tile([128, 512], F32, tag="y")
                    for k in range(6):
                        nc.tensor.matmul(
                            y,
                            wt_s[:, k, fb * 128:(fb + 1) * 128],
                            xt_t[:, k, :],
                            start=(k == 0),
                            stop=(k == 5),
                        )
                    s_t = sbw.tile([128, 512], F32, tag="s")
                    nc.scalar.activation(s_t, y, Act.Sin, bias=ph_s[:, fb:fb + 1], scale=1.0)
                    t_t = sbw.tile([128, 512], F32R, tag="t")
                    nc.scalar.activation(t_t, s_t, Act.Tanh, bias=0.0, scale=0.5)

                    if fi == 0:
                        flush_tail()  # prev iteration's transposes/copies/DMA

                    val = ps_v.tile([128, 512], F32, tag="val")
                    t_in = t_t.bitcast(F32)
                    tks = []
                    for j, gp in enumerate(GPRIME):
                        tk = tkp.tile([128, 512], BF16 if MAIN_BF16 else F32R, tag=f"tk{j}")
                        dj = j * 4 + fb if j < 6 else 24 + fb
                        if j in ACT_BINS:
                            nc.scalar.activation(
                                tk, t_in, Act.Relu,
                                bias=nb_sc[:, dj:dj + 1], scale=wb_sc[:, dj:dj + 1],
                            )
                        elif j in GPSIMD_BINS:
                            nc.gpsimd.tensor_scalar(
                                tk, t_in, gp, wb_sc[:, dj:dj + 1], Alu.max, Alu.mult
                            )
                        else:
                            nc.vector.tensor_scalar(
                                tk, t_in, gp, wb_sc[:, dj:dj + 1], Alu.max, Alu.mult
                            )
                        tks.append(tk)
                    spl_id = identb_s if MAIN_BF16 else ident_s
                    for j, tk in enumerate(tks):
                        nc.tensor.matmul(
                            val, spl_id, tk,
                            start=(j == 0), stop=(j == len(tks) - 1),
                        )
                    vals.append(val)

                # rope for this pair: features p = pb*128 + [0..127]
                va, vb = vals
                c_ap = cos_s[:, pb, mb * 512:(mb + 1) * 512]
                s_ap = sin_s[:, pb, mb * 512:(mb + 1) * 512]
                ba = bb_s[:, fba:fba + 1]
                bb_ = bb_s[:, fbb:fbb + 1]
                m1 = rop.tile([128, 512], F32, tag="m1")
                m2 = rop.tile([128, 512], F32, tag="m2")
                m3 = rop.tile([128, 512], F32, tag="m3")
                m4 = rop.tile([128, 512], F32, tag="m4")
                nc.vector.scalar_tensor_tensor(m1, va, ba, c_ap, Alu.add, Alu.mult)
                nc.vector.scalar_tensor_tensor(m2, vb, bb_, s_ap, Alu.add, Alu.mult)
                nc.vector.scalar_tensor_tensor(m3, va, ba, s_ap, Alu.add, Alu.mult)
                nc.vector.scalar_tensor_tensor(m4, vb, bb_, c_ap, Alu.add, Alu.mult)
                re = rop.tile([128, 512], F32R, tag="re")
                ro = rop.tile([128, 512], F32R, tag="ro")
                if GPSIMD_ROPE_TAIL:
                    nc.gpsimd.tensor_tensor(re, m1, m2, Alu.subtract)
                    nc.gpsimd.tensor_tensor(ro, m3, m4, Alu.add)
                else:
                    nc.vector.tensor_sub(re, m1, m2)
                    nc.vector.tensor_add(ro, m3, m4)

                def make_tail(mb=mb, pb=pb, re=re, ro=ro, out_ts=out_ts):
                    def tail():
                        # transpose + interleave:  out[m, 2p + parity]
                        for h in range(2):
                            po = ps_o.tile([128, 512], F32R, tag="po")
                            for q in range(2):
                                mc = h * 2 + q
                                nc.tensor.transpose(
                                    po[:, q * 256:q * 256 + 128],
                                    re[:, mc * 128:(mc + 1) * 128],
                                    ident_s,
                                )
                                nc.tensor.transpose(
                                    po[:, q * 256 + 128:q * 256 + 256],
                                    ro[:, mc * 128:(mc + 1) * 128],
                                    ident_s,
                                )
                            for q in range(2):
                                mc = h * 2 + q
                                src = po.bitcast(F32)[:, q * 256:(q + 1) * 256].rearrange(
                                    "p (two x) -> p two x", two=2
                                )
                                dst = out_ts[mc][:, pb * 256:(pb + 1) * 256].rearrange(
                                    "p (x two) -> p two x", two=2
                                )
                                nc.scalar.copy(dst, src)
                        if pb == 1:
                            for mc in range(4):
                                nc.sync.dma_start(
                                    out=out[(mb * 4 + mc) * 128:(mb * 4 + mc + 1) * 128, :],
                                    in_=out_ts[mc],
                                )

                    return tail

                pending.append(make_tail())

            flush_tail()

    try:
        nc.compile()
    finally:
        hw_specs.get_activation_tables = _orig_tables
        bacc.get_activation_tables = _orig_tables
    return nc


ACT_BINS_HOST = None


def _host_prep(x, perm_freqs, perm_phase, spline_heights, spline_bias, offset):
    """Derive all device inputs on the host (cheap, O(DIM*IN_DIM))."""
    x = np.asarray(x, dtype=np.float32)
    W = np.asarray(perm_freqs, dtype=np.float32)
    phase = np.asarray(perm_phase, dtype=np.float32)[:, 0]
    heights = np.asarray(spline_heights, dtype=np.float32)
    bias = np.asarray(spline_bias, dtype=np.float32)
    offset = int(np.asarray(offset))

    perm = np.concatenate([np.arange(0, DIM, 2), np.arange(1, DIM, 2)])
    Wp = W[perm]
    php = phase[perm]
    hp = heights[perm].astype(np.float64)
    bp = bias[perm].astype(np.float64)

    w = np.log1p(np.exp(hp))  # softplus, [512, 16]
    g = np.linspace(0.0, 1.0, NUM_BINS)
    A = w[:, :5].sum(axis=1)
    C = (w[:, :5] * g[:5]).sum(axis=1)
    wb = 0.5 * w[:, 5:11]                      # [512, 6]
    gp = 2.0 * g[5:11] - 1.0                   # [6]
    ab = 0.5 * A
    # ACT-routed bins use the relu form w*max(t,g') - w*g' (their constant is
    # inside the op), so bbar only carries -w*g' for the DVE max-form bins.
    dve_mask = np.array([j not in ACT_BINS for j in range(6)])
    bb = 0.5 * A - C + bp - (wb[:, dve_mask] * gp[dve_mask]).sum(axis=1)

    wbar = np.zeros((128, 28), dtype=np.float32)
    nbias = np.zeros((128, 28), dtype=np.float32)
    for j in range(6):
        for fb in range(4):
            wbar[:, j * 4 + fb] = wb[fb * 128:(fb + 1) * 128, j]
            nbias[:, j * 4 + fb] = -(wb[fb * 128:(fb + 1) * 128, j] * gp[j])
    for fb in range(4):
        wbar[:, 24 + fb] = ab[fb * 128:(fb + 1) * 128]

    idx = np.arange(N, dtype=np.float64) + offset
    days = np.floor(idx / DAY_LENGTH)
    hours = np.mod(idx, DAY_LENGTH)
    half = np.arange(0, DIM, 2, dtype=np.float64) / DIM
    inv_h = 1.0 / (10000.0 ** half)
    inv_d = 1.0 / (100000.0 ** half)
    ang = hours[:, None] * inv_h + days[:, None] * inv_d    # [N, 256]
    cosT = np.cos(ang).T.astype(np.float32).copy()          # [256, N]
    sinT = np.sin(ang).T.astype(np.float32).copy()

    mdt = ml_dtypes_bfloat16() if MAIN_BF16 else np.float32
    shared = dict(
        wt=np.ascontiguousarray(
            Wp.T.reshape(6, 128, DIM).transpose(1, 0, 2)
        ).astype(mdt),                                       # [128, 6, 512]
        scal=np.concatenate(
            [wbar, nbias,
             php.astype(np.float32).reshape(4, 128).T,
             bb.astype(np.float32).reshape(4, 128).T],
            axis=1,
        ),
        ident=np.eye(128, dtype=np.float32),
        identb=np.eye(128, dtype=np.float32).astype(ml_dtypes_bfloat16()),
        cosT=cosT,
        sinT=sinT,
    )
    xts = [
        np.ascontiguousarray(x[c].T.reshape(6, 128, N).transpose(1, 0, 2)).astype(mdt)
        for c in range(B)
    ]   # [128, 6, 2048] each
    return shared, xts


def _enable_ldw_opt():
    import concourse.bass_utils as bu

    if getattr(bu, "_ldw_opt_patched", False):
        return
    orig = bu.run_command

    def patched(argv, **kw):
        argv = ["--enable-ldw-opt=true" if a == "--enable-ldw-opt=false" else a for a in argv]
        return orig(argv, **kw)

    bu.run_command = patched
    bu._ldw_opt_patched = True


def kernel(x, perm_freqs, perm_phase, spline_heights, spline_bias, offset):
    from concourse.bass_utils import run_bass_kernel_spmd

    if not MAIN_BF16:
        _enable_ldw_opt()

    if "nc" not in _CACHE:
        _CACHE["nc"] = _build()
    nc = _CACHE["nc"]

    shared, xts = _host_prep(x, perm_freqs, perm_phase, spline_heights, spline_bias, offset)
    in_maps = [dict(shared, xt=xts[c]) for c in range(NCORES)]
    kw = {}
    if TRACE:
        import tempfile

        kw = dict(trace=True, tmpdir=tempfile.mkdtemp(prefix="nucleus_trace_"))
        _CACHE["trace_dir"] = kw["tmpdir"]
    r = run_bass_kernel_spmd(nc, in_maps, core_ids=list(range(NCORES)), **kw)
    out = np.stack([r.results[c]["out"] for c in range(NCORES)], axis=0)
    _CACHE["last_exec_time_ns"] = r.exec_time_ns
    return out


# revision 22
# speedup vs baseline: 1.1581x; 1.1581x over previous
"""Trainium2 Bass kernel for nn_Atom_57732950393048 (Nucleus MLP + RoPE).

Math (per batch b, all features f, tokens n):
    y = x @ W^T + phase                      # [N, 512], W = perm_freqs
    s = sin(y)            in [-1, 1]
    u = sigmoid(s)        in [0.2689, 0.7311]
    val = sum_k w_k relu(u - k/15) + bias,   w = softplus(spline_heights)
    out = rope(val)

Because u is confined to [sigmoid(-1), sigmoid(1)], spline bins 0..4 are
always active (linear) and bins 11..15 always inactive.  With the tanh form
u = 0.5 + 0.5 t, t = tanh(s/2):
    val = sum_{k=5..10} wb_k * max(t, g'_k) + ab * t + bb
with wb_k = w_k/2, g'_k = 2k/15 - 1, ab = sum_{k<=4} w_k / 2, and all the
constants folded into bb.  max(t, g'_k) == max(t, -1) == t for the linear
term, so the linear term is just one more diagonal matmul.

Device layout (one batch per core, 8 cores, data parallel):
  - orientation "B": features on partitions (4 blocks of 128), tokens on the
    free dim.  Main matmul: out[f, m] += WT_chunk.T @ XT_chunk, f32r.
  - spline accumulation: PE diagonal matmuls  val[f, m] += diag(wb_k) @ t_k
    accumulated in PSUM (t_k = max(t, g'_k) computed by DVE tensor_scalar).
  - RoPE applied in orientation B on feature-PERMUTED channels (evens then
    odds, so rope partners are partition blocks (0,2) and (1,3)); the
    per-feature constant bb is folded into the rope multiplies via
    scalar_tensor_tensor: (val + bb) * table.
  - PE transposes (f32r) flip [f, m] -> [m, f] and ACT copies interleave
    even/odd channels into the final [m, 512] layout, then DMA to DRAM.
"""

import numpy as np


def ml_dtypes_bfloat16():
    import ml_dtypes

    return ml_dtypes.bfloat16


NUM_BINS = 16
DAY_LENGTH = 64
B, N, IN_DIM, DIM = 8, 2048, 768, 512
NCORES = 8

_CACHE = {}
TRACE = False
MAIN_BF16 = False
ACT_BINS = (0, 1)      # bins on ScalarE (relu form)
GPSIMD_BINS = ()       # bins on GpSimd (measured: disastrous)
GPSIMD_ROPE_TAIL = False


def _build():
    import concourse.bacc as bacc
    import concourse.tile as tile
    from concourse import mybir

    # Pin all our activation funcs to the single table set that holds every
    # one of them (silu_and_others: sin + tanh + relu + copy + identity) by
    # hiding those funcs from every other set in the selection table.  Set
    # ids are positional, so membership may be edited but never reordered.
    import concourse.hw_specs as hw_specs

    _orig_tables = hw_specs.get_activation_tables

    def _pinned_tables(arch):
        t = _orig_tables(arch)
        A = mybir.ActivationFunctionType
        shared = {A.Sin, A.Tanh, A.Copy, A.Identity, A.Relu}
        if "silu_and_others" in t and shared <= t["silu_and_others"]:
            for name in t:
                if name != "silu_and_others":
                    t[name] = t[name] - shared
        return t

    hw_specs.get_activation_tables = _pinned_tables
    bacc.get_activation_tables = _pinned_tables

    F32 = mybir.dt.float32
    F32R = mybir.dt.float32r
    Alu = mybir.AluOpType
    Act = mybir.ActivationFunctionType

    nc = bacc.Bacc(trn_type="TRN2")

    BF16 = mybir.dt.bfloat16
    MDT = BF16 if MAIN_BF16 else F32R
    xt = nc.dram_tensor("xt", [128, 6, N], MDT, kind="ExternalInput")
    wt = nc.dram_tensor("wt", [128, 6, DIM], MDT, kind="ExternalInput")
    scal = nc.dram_tensor("scal", [128, 64], F32, kind="ExternalInput")
    ident = nc.dram_tensor("ident", [128, 128], F32R, kind="ExternalInput")
    identb = nc.dram_tensor("identb", [128, 128], BF16, kind="ExternalInput")
    cosT = nc.dram_tensor("cosT", [256, N], F32, kind="ExternalInput")
    sinT = nc.dram_tensor("sinT", [256, N], F32, kind="ExternalInput")
    out = nc.dram_tensor("out", [N, DIM], F32, kind="ExternalOutput")

    # bin breakpoints in t-space; the 7th entry (-1.0) implements the linear
    # term: max(t, -1) == t.
    GPRIME = [2.0 * k / 15.0 - 1.0 for k in range(5, 11)] + [-1.0]

    MBLK = N // 512  # 4 m-blocks of 512 tokens

    def bass_ap_chunks(ap, nk, width):
        # [nk*128, width] DRAM view -> [128 p, nk, width] (partition-major)
        return ap.rearrange("(k p) w -> p k w", p=128)

    with tile.TileContext(nc) as tc:
        from contextlib import ExitStack

        with ExitStack() as ctx:
            res = ctx.enter_context(tc.tile_pool(name="res", bufs=1))
            xtp = ctx.enter_context(tc.tile_pool(name="xtp", bufs=2))
            sbw = ctx.enter_context(tc.tile_pool(name="sbw", bufs=4))
            tkp = ctx.enter_context(tc.tile_pool(name="tkp", bufs=3))
            rop = ctx.enter_context(tc.tile_pool(name="rop", bufs=3))
            outp = ctx.enter_context(tc.tile_pool(name="outp", bufs=2))
            ps_y = ctx.enter_context(tc.tile_pool(name="ps_y", bufs=3, space="PSUM"))
            ps_v = ctx.enter_context(tc.tile_pool(name="ps_v", bufs=3, space="PSUM"))
            ps_o = ctx.enter_context(tc.tile_pool(name="ps_o", bufs=2, space="PSUM"))

            # --- resident tensors.  DMA order is startup-latency-critical:
            # the first k-chunks of x and W go first (the first matmuls gate
            # on them), then the merged scalar tensor, identity, tables. ---
            wt_s = res.tile([128, 6, DIM], MDT, tag="wt")
            scal_s = res.tile([128, 64], F32, tag="scal")
            wb_sc = scal_s[:, 0:28]
            nb_sc = scal_s[:, 28:56]
            ph_s = scal_s[:, 56:60]
            bb_s = scal_s[:, 60:64]
            ident_s = res.tile([128, 128], F32R, tag="ident")
            identb_s = res.tile([128, 128], BF16, tag="identb")
            cos_s = res.tile([128, 2, N], F32, tag="cos")
            sin_s = res.tile([128, 2, N], F32, tag="sin")

            # one-iteration software pipeline: the transpose/interleave/DMA
            # tail of iteration i is emitted after iteration i+1's first main
            # matmul group so PE never drains while DVE finishes the rope.
            pending = []

            def flush_tail():
                for fn in pending:
                    fn()
                pending.clear()

            mb_out_ts = {}
            for it in range(2 * MBLK):
                mb, pb = divmod(it, 2)
                fba, fbb = (0, 2) if pb == 0 else (1, 3)

                if pb == 0:
                    xt_t = xtp.tile([128, 6, 512], MDT, tag="xt")
                    if mb == 0:
                        nc.sync.dma_start(out=wt_s, in_=wt[:])
                        nc.sync.dma_start(out=scal_s, in_=scal[:])
                        for k in range(6):
                            nc.sync.dma_start(out=xt_t[:, k, :], in_=xt[:, k, 0:512])
                        nc.sync.dma_start(out=ident_s, in_=ident[:])
                        nc.sync.dma_start(out=identb_s, in_=identb[:])
                        nc.sync.dma_start(out=cos_s[:, 0, :], in_=cosT[0:128, :])
                        nc.sync.dma_start(out=sin_s[:, 0, :], in_=sinT[0:128, :])
                    else:
                        nc.sync.dma_start(
                            out=xt_t, in_=xt[:, :, mb * 512:(mb + 1) * 512]
                        )
                    mb_out_ts[mb] = []
                    for mc in range(4):
                        o_t = outp.tile([128, DIM], F32, tag=f"out{mc}")
                        mb_out_ts[mb].append(o_t)
                    if mb:
                        mb_out_ts.pop(mb - 1)
                    xt_cur = xt_t
                else:
                    xt_t = xt_cur
                    if mb == 0:
                        # pair-1 tables load during pair-0 compute
                        nc.sync.dma_start(out=cos_s[:, 1, :], in_=cosT[128:256, :])
                        nc.sync.dma_start(out=sin_s[:, 1, :], in_=sinT[128:256, :])
                out_ts = mb_out_ts[mb]

                vals = []
                for fi, fb in enumerate((fba, fbb)):
                    y = ps_y.tile([128, 512], F32, tag="y")
                    for k in range(6):
                        nc.tensor.matmul(
                            y,
                            wt_s[:, k, fb * 128:(fb + 1) * 128],
                            xt_t[:, k, :],
                            start=(k == 0),
                            stop=(k == 5),
                        )
                    s_t = sbw.tile([128, 512], F32, tag="s")
                    nc.scalar.activation(s_t, y, Act.Sin, bias=ph_s[:, fb:fb + 1], scale=1.0)
                    t_t = sbw.tile([128, 512], F32R, tag="t")
                    nc.scalar.activation(t_t, s_t, Act.Tanh, bias=0.0, scale=0.5)

                    if fi == 0:
                        flush_tail()  # prev iteration's transposes/copies/DMA

                    val = ps_v.tile([128, 512], F32, tag="val")
                    t_in = t_t.bitcast(F32)
                    tks = []
                    for j, gp in enumerate(GPRIME):
                        tk = tkp.tile([128, 512], BF16 if MAIN_BF16 else F32R, tag=f"tk{j}")
                        dj = j * 4 + fb if j < 6 else 24 + fb
                        if j in ACT_BINS:
                            nc.scalar.activation(
                                tk, t_in, Act.Relu,
                                bias=nb_sc[:, dj:dj + 1], scale=wb_sc[:, dj:dj + 1],
                            )
                        elif j in GPSIMD_BINS:
                            nc.gpsimd.tensor_scalar(
                                tk, t_in, gp, wb_sc[:, dj:dj + 1], Alu.max, Alu.mult
                            )
                        else:
                            nc.vector.tensor_scalar(
                                tk, t_in, gp, wb_sc[:, dj:dj + 1], Alu.max, Alu.mult
                            )
                        tks.append(tk)
                    spl_id = identb_s if MAIN_BF16 else ident_s
                    for j, tk in enumerate(tks):
                        nc.tensor.matmul(
                            val, spl_id, tk,
                            start=(j == 0), stop=(j == len(tks) - 1),
                        )
                    vals.append(val)

                # rope for this pair: features p = pb*128 + [0..127]
                va, vb = vals
                c_ap = cos_s[:, pb, mb * 512:(mb + 1) * 512]
                s_ap = sin_s[:, pb, mb * 512:(mb + 1) * 512]
                ba = bb_s[:, fba:fba + 1]
                bb_ = bb_s[:, fbb:fbb + 1]
                m1 = rop.tile([128, 512], F32, tag="m1")
                m2 = rop.tile([128, 512], F32, tag="m2")
                m3 = rop.tile([128, 512], F32, tag="m3")
                m4 = rop.tile([128, 512], F32, tag="m4")
                nc.vector.scalar_tensor_tensor(m1, va, ba, c_ap, Alu.add, Alu.mult)
                nc.vector.scalar_tensor_tensor(m2, vb, bb_, s_ap, Alu.add, Alu.mult)
                nc.vector.scalar_tensor_tensor(m3, va, ba, s_ap, Alu.add, Alu.mult)
                nc.vector.scalar_tensor_tensor(m4, vb, bb_, c_ap, Alu.add, Alu.mult)
                re = rop.tile([128, 512], F32R, tag="re")
                ro = rop.tile([128, 512], F32R, tag="ro")
                if GPSIMD_ROPE_TAIL:
                    nc.gpsimd.tensor_tensor(re, m1, m2, Alu.subtract)
                    nc.gpsimd.tensor_tensor(ro, m3, m4, Alu.add)
                else:
                    nc.vector.tensor_sub(re, m1, m2)
                    nc.vector.tensor_add(ro, m3, m4)

                def make_tail(mb=mb, pb=pb, re=re, ro=ro, out_ts=out_ts):
                    def tail():
                        # transpose + interleave:  out[m, 2p + parity]
                        for h in range(2):
                            po = ps_o.tile([128, 512], F32R, tag="po")
                            for q in range(2):
                                mc = h * 2 + q
                                nc.tensor.transpose(
                                    po[:, q * 256:q * 256 + 128],
                                    re[:, mc * 128:(mc + 1) * 128],
                                    ident_s,
                                )
                                nc.tensor.transpose(
                                    po[:, q * 256 + 128:q * 256 + 256],
                                    ro[:, mc * 128:(mc + 1) * 128],
                                    ident_s,
                                )
                            for q in range(2):
                                mc = h * 2 + q
                                src = po.bitcast(F32)[:, q * 256:(q + 1) * 256].rearrange(
                                    "p (two x) -> p two x", two=2
                                )
                                dst = out_ts[mc][:, pb * 256:(pb + 1) * 256].rearrange(
                                    "p (x two) -> p two x", two=2
                                )
                                nc.scalar.copy(dst, src)
                        if pb == 1:
                            for mc in range(4):
                                nc.sync.dma_start(
                                    out=out[(mb * 4 + mc) * 128:(mb * 4 + mc + 1) * 128, :],
                                    in_=out_ts[mc],
                                )

                    return tail

                pending.append(make_tail())

            flush_tail()

    try:
        nc.compile()
    finally:
        hw_specs.get_activation_tables = _orig_tables
        bacc.get_activation_tables = _orig_tables
    return nc


ACT_BINS_HOST = None


def _host_prep(x, perm_freqs, perm_phase, spline_heights, spline_bias, offset):
    """Derive all device inputs on the host (cheap, O(DIM*IN_DIM))."""
    x = np.asarray(x, dtype=np.float32)
    W = np.asarray(perm_freqs, dtype=np.float32)
    phase = np.asarray(perm_phase, dtype=np.float32)[:, 0]
    heights = np.asarray(spline_heights, dtype=np.float32)
    bias = np.asarray(spline_bias, dtype=np.float32)
    offset = int(np.asarray(offset))

    perm = np.concatenate([np.arange(0, DIM, 2), np.arange(1, DIM, 2)])
    Wp = W[perm]
    php = phase[perm]
    hp = heights[perm].astype(np.float64)
    bp = bias[perm].astype(np.float64)

    w = np.log1p(np.exp(hp))  # softplus, [512, 16]
    g = np.linspace(0.0, 1.0, NUM_BINS)
    A = w[:, :5].sum(axis=1)
    C = (w[:, :5] * g[:5]).sum(axis=1)
    wb = 0.5 * w[:, 5:11]                      # [512, 6]
    gp = 2.0 * g[5:11] - 1.0                   # [6]
    ab = 0.5 * A
    # ACT-routed bins use the relu form w*max(t,g') - w*g' (their constant is
    # inside the op), so bbar only carries -w*g' for the DVE max-form bins.
    dve_mask = np.array([j not in ACT_BINS for j in range(6)])
    bb = 0.5 * A - C + bp - (wb[:, dve_mask] * gp[dve_mask]).sum(axis=1)

    wbar = np.zeros((128, 28), dtype=np.float32)
    nbias = np.zeros((128, 28), dtype=np.float32)
    for j in range(6):
        for fb in range(4):
            wbar[:, j * 4 + fb] = wb[fb * 128:(fb + 1) * 128, j]
            nbias[:, j * 4 + fb] = -(wb[fb * 128:(fb + 1) * 128, j] * gp[j])
    for fb in range(4):
        wbar[:, 24 + fb] = ab[fb * 128:(fb + 1) * 128]

    idx = np.arange(N, dtype=np.float64) + offset
    days = np.floor(idx / DAY_LENGTH)
    hours = np.mod(idx, DAY_LENGTH)
    half = np.arange(0, DIM, 2, dtype=np.float64) / DIM
    inv_h = 1.0 / (10000.0 ** half)
    inv_d = 1.0 / (100000.0 ** half)
    ang = hours[:, None] * inv_h + days[:, None] * inv_d    # [N, 256]
    cosT = np.cos(ang).T.astype(np.float32).copy()          # [256, N]
    sinT = np.sin(ang).T.astype(np.float32).copy()

    mdt = ml_dtypes_bfloat16() if MAIN_BF16 else np.float32
    shared = dict(
        wt=np.ascontiguousarray(
            Wp.T.reshape(6, 128, DIM).transpose(1, 0, 2)
        ).astype(mdt),                                       # [128, 6, 512]
        scal=np.concatenate(
            [wbar, nbias,
             php.astype(np.float32).reshape(4, 128).T,
             bb.astype(np.float32).reshape(4, 128).T],
            axis=1,
        ),
        ident=np.eye(128, dtype=np.float32),
        identb=np.eye(128, dtype=np.float32).astype(ml_dtypes_bfloat16()),
        cosT=cosT,
        sinT=sinT,
    )
    xts = [
        np.ascontiguousarray(x[c].T.reshape(6, 128, N).transpose(1, 0, 2)).astype(mdt)
        for c in range(B)
    ]   # [128, 6, 2048] each
    return shared, xts


def _enable_ldw_opt():
    import concourse.bass_utils as bu

    if getattr(bu, "_ldw_opt_patched", False):
        return
    orig = bu.run_command

    def patched(argv, **kw):
        argv = ["--enable-ldw-opt=true" if a == "--enable-ldw-opt=false" else a for a in argv]
        return orig(argv, **kw)

    bu.run_command = patched
    bu._ldw_opt_patched = True


def kernel(x, perm_freqs, perm_phase, spline_heights, spline_bias, offset):
    from concourse.bass_utils import run_bass_kernel_spmd

    if not MAIN_BF16:
        _enable_ldw_opt()

    if "nc" not in _CACHE:
        _CACHE["nc"] = _build()
    nc = _CACHE["nc"]

    shared, xts = _host_prep(x, perm_freqs, perm_phase, spline_heights, spline_bias, offset)
    in_maps = [dict(shared, xt=xts[c]) for c in range(NCORES)]
    kw = {}
    if TRACE:
        import tempfile

        kw = dict(trace=True, tmpdir=tempfile.mkdtemp(prefix="nucleus_trace_"))
        _CACHE["trace_dir"] = kw["tmpdir"]
    r = run_bass_kernel_spmd(nc, in_maps, core_ids=list(range(NCORES)), **kw)
    out = np.stack([r.results[c]["out"] for c in range(NCORES)], axis=0)
    _CACHE["last_exec_time_ns"] = r.exec_time_ns
    return out
